# revision 5
# baseline (speedup 1.0000x reference)
"""SATD-style custom loss on 8 Trainium2 NeuronCores.

Computes sum(|H8 @ (original - pred)|) where H8 is the 8x8 Sylvester
Hadamard matrix applied along dim -2 of [B, C, 8, 8] blocks.

Strategy: pure data parallel over the block-batch dim (8 shards).
Per core:
  - gpsimd DMA loads with inline fp32->bf16 cast (halves on-chip traffic)
  - DVE: diff + 3-stage fast Walsh-Hadamard transform along j
    (butterfly distances 8/16/32 elements within each 64-elem block)
  - ACT: fused Abs + per-partition accumulate (accum_out)
  - final DVE reduce to [128,1] per core; host sums 8x128 partials.
"""

import numpy as np

import concourse.bacc as bacc
import concourse.bass as bass
import concourse.mybir as mybir
from concourse.bass_utils import run_bass_kernel_spmd
from concourse.tile import TileContext

# Problem shape (hardcoded; kernel.py must be self-contained).
N_BLOCKS = 524288
C = 3
N_CORES = 8
ELEMS_PER_CORE = (N_BLOCKS // N_CORES) * C * 64  # 12_582_912
P = 128  # SBUF partitions
F = 4096  # fp32 elems per partition per tile
ROWS = ELEMS_PER_CORE // F  # 3072
T = ROWS // P  # 24 tiles per core
NB = F // 64  # 64 SATD blocks per partition per tile

CAST_ON_DMA = True  # fp32->bf16 during DMA (SWDGE); else cast in the diff op


def _build_program() -> bass.Bass:
    nc = bacc.Bacc("TRN2", debug=False, num_devices=N_CORES)
    dt = mybir.dt

    o_dram = nc.declare_dram_parameter("o", [ROWS, F], dt.float32, isOutput=False)
    p_dram = nc.declare_dram_parameter("p", [ROWS, F], dt.float32, isOutput=False)
    out_dram = nc.declare_dram_parameter("out", [P, 1], dt.float32, isOutput=True)

    in_dt = dt.bfloat16 if CAST_ON_DMA else dt.float32

    with TileContext(nc) as tc:
        with (
            tc.tile_pool(name="io", bufs=3) as io_pool,
            tc.tile_pool(name="work", bufs=3) as work_pool,
            tc.tile_pool(name="acc", bufs=1) as acc_pool,
        ):
            acc = acc_pool.tile([P, T], dt.float32)

            for t in range(T):
                ob = io_pool.tile([P, F], in_dt, tag="ob")
                pb = io_pool.tile([P, F], in_dt, tag="pb")
                dma_o = nc.gpsimd if CAST_ON_DMA else nc.sync
                dma_o.dma_start(out=ob[:], in_=o_dram[t * P : (t + 1) * P, :])
                dma_o.dma_start(out=pb[:], in_=p_dram[t * P : (t + 1) * P, :])

                # diff (cast to bf16 on write when loads are fp32)
                d0 = work_pool.tile([P, F], dt.bfloat16, tag="d0")
                nc.vector.tensor_sub(d0[:], ob[:], pb[:])

                # FWHT along j: free offset within a block = j*8 + w.
                # stage 1: combine j-bit0 (element distance 8)
                d1 = work_pool.tile([P, F], dt.bfloat16, tag="d1")
                v0 = d0[:].rearrange("p (b j2 s w) -> p b j2 s w", j2=4, s=2, w=8)
                v1 = d1[:].rearrange("p (b j2 s w) -> p b j2 s w", j2=4, s=2, w=8)
                nc.vector.tensor_add(v1[:, :, :, 0, :], v0[:, :, :, 0, :], v0[:, :, :, 1, :])
                nc.vector.tensor_sub(v1[:, :, :, 1, :], v0[:, :, :, 0, :], v0[:, :, :, 1, :])

                # stage 2: combine j-bit1 (element distance 16)
                d2 = work_pool.tile([P, F], dt.bfloat16, tag="d2")
                w1 = d1[:].rearrange("p (b jh s jl) -> p b jh s jl", jh=2, s=2, jl=16)
                w2 = d2[:].rearrange("p (b jh s jl) -> p b jh s jl", jh=2, s=2, jl=16)
                nc.vector.tensor_add(w2[:, :, :, 0, :], w1[:, :, :, 0, :], w1[:, :, :, 1, :])
                nc.vector.tensor_sub(w2[:, :, :, 1, :], w1[:, :, :, 0, :], w1[:, :, :, 1, :])

                # stage 3: combine j-bit2 (element distance 32)
                d3 = work_pool.tile([P, F], dt.bfloat16, tag="d3")
                x2 = d2[:].rearrange("p (b s jl) -> p b s jl", s=2, jl=32)
                x3 = d3[:].rearrange("p (b s jl) -> p b s jl", s=2, jl=32)
                nc.vector.tensor_add(x3[:, :, 0, :], x2[:, :, 0, :], x2[:, :, 1, :])
                nc.vector.tensor_sub(x3[:, :, 1, :], x2[:, :, 0, :], x2[:, :, 1, :])

                # abs + per-partition running sum for this tile (ACT engine)
                d4 = work_pool.tile([P, F], dt.bfloat16, tag="d4")
                nc.scalar.activation(
                    out=d4[:],
                    in_=d3[:],
                    func=mybir.ActivationFunctionType.Abs,
                    accum_out=acc[:, t : t + 1],
                )

            accsum = acc_pool.tile([P, 1], dt.float32)
            nc.vector.tensor_reduce(
                out=accsum[:],
                in_=acc[:],
                axis=mybir.AxisListType.X,
                op=mybir.AluOpType.add,
            )
            nc.sync.dma_start(out=out_dram[:, :], in_=accsum[:])

    nc.compile()
    return nc


_NC_CACHE: bass.Bass | None = None


def _get_program() -> bass.Bass:
    global _NC_CACHE
    if _NC_CACHE is None:
        _NC_CACHE = _build_program()
    return _NC_CACHE


def run(original: np.ndarray, pred: np.ndarray, trace: bool = False, **kwargs):
    """Shard, run on 8 cores, return (scalar result, BassKernelResults)."""
    o = np.ascontiguousarray(np.asarray(original, dtype=np.float32)).reshape(
        N_CORES, ROWS, F
    )
    p = np.ascontiguousarray(np.asarray(pred, dtype=np.float32)).reshape(
        N_CORES, ROWS, F
    )
    in_maps = [{"o": o[i], "p": p[i]} for i in range(N_CORES)]
    nc = _get_program()
    res = run_bass_kernel_spmd(
        nc, in_maps, core_ids=list(range(N_CORES)), trace=trace, **kwargs
    )
    total = np.float64(0.0)
    for r in res.results:
        total += r["out"].astype(np.float64).sum()
    return np.array(total, dtype=np.float32), res


def kernel(original: np.ndarray, pred: np.ndarray) -> np.ndarray:
    out, _ = run(original, pred, trace=False)
    return out


# revision 11
# speedup vs baseline: 1.2400x; 1.2400x over previous
"""SATD-style custom loss on 8 Trainium2 NeuronCores.

Computes sum(|H8 @ (original - pred)|) where H8 is the 8x8 Sylvester
Hadamard matrix applied along dim -2 of [B, C, 8, 8] blocks.

Strategy: pure data parallel over the block-batch dim (8 shards).
Per core:
  - gpsimd DMA loads with inline fp32->bf16 cast (halves on-chip traffic)
  - DVE: diff + 3-stage fast Walsh-Hadamard transform along j
    (butterfly distances 8/16/32 elements within each 64-elem block)
  - ACT: fused Abs + per-partition accumulate (accum_out)
  - final DVE reduce to [128,1] per core; host sums 8x128 partials.
"""

import numpy as np

import concourse.bacc as bacc
import concourse.bass as bass
import concourse.mybir as mybir
from concourse.bass_utils import run_bass_kernel_spmd
from concourse.tile import TileContext

# Problem shape (hardcoded; kernel.py must be self-contained).
N_BLOCKS = 524288
C = 3
N_CORES = 8
ELEMS_PER_CORE = (N_BLOCKS // N_CORES) * C * 64  # 12_582_912
P = 128  # SBUF partitions
F = 4096  # fp32 elems per partition per tile
ROWS = ELEMS_PER_CORE // F  # 3072
T = ROWS // P  # 24 tiles per core
NB = F // 64  # 64 SATD blocks per partition per tile

CAST_ON_DMA = True  # fp32->bf16 during DMA (SWDGE); else cast in the diff op


def _build_program() -> bass.Bass:
    nc = bacc.Bacc("TRN2", debug=False, num_devices=N_CORES)
    dt = mybir.dt

    # Host interleaves o|p per row: x[r] = [o_row_r (F), p_row_r (F)].
    # One DMA per tile -> in-order tile completion, single dep for the diff.
    x_dram = nc.declare_dram_parameter("x", [ROWS, 2 * F], dt.float32, isOutput=False)
    out_dram = nc.declare_dram_parameter("out", [P, 1], dt.float32, isOutput=True)

    in_dt = dt.bfloat16 if CAST_ON_DMA else dt.float32

    with TileContext(nc) as tc:
        with (
            tc.tile_pool(name="io", bufs=4) as io_pool,
            tc.tile_pool(name="work", bufs=3) as work_pool,
            tc.tile_pool(name="acc", bufs=1) as acc_pool,
        ):
            acc = acc_pool.tile([P, T], dt.float32)

            for t in range(T):
                dma_eng = nc.gpsimd if CAST_ON_DMA else nc.sync
                xb = io_pool.tile([P, 2 * F], in_dt, tag="xb")
                dma_eng.dma_start(out=xb[:], in_=x_dram[t * P : (t + 1) * P, :])

                # diff of the o-half and p-half
                d0 = work_pool.tile([P, F], dt.bfloat16, tag="d0")
                nc.vector.tensor_sub(d0[:], xb[:, 0:F], xb[:, F : 2 * F])

                # FWHT along j: free offset within a block = j*8 + w.
                # stage 1: combine j-bit0 (element distance 8)
                d1 = work_pool.tile([P, F], dt.bfloat16, tag="d1")
                v0 = d0[:].rearrange("p (b j2 s w) -> p b j2 s w", j2=4, s=2, w=8)
                v1 = d1[:].rearrange("p (b j2 s w) -> p b j2 s w", j2=4, s=2, w=8)
                nc.vector.tensor_add(v1[:, :, :, 0, :], v0[:, :, :, 0, :], v0[:, :, :, 1, :])
                nc.vector.tensor_sub(v1[:, :, :, 1, :], v0[:, :, :, 0, :], v0[:, :, :, 1, :])

                # stage 2: combine j-bit1 (element distance 16)
                d2 = work_pool.tile([P, F], dt.bfloat16, tag="d2")
                w1 = d1[:].rearrange("p (b jh s jl) -> p b jh s jl", jh=2, s=2, jl=16)
                w2 = d2[:].rearrange("p (b jh s jl) -> p b jh s jl", jh=2, s=2, jl=16)
                nc.vector.tensor_add(w2[:, :, :, 0, :], w1[:, :, :, 0, :], w1[:, :, :, 1, :])
                nc.vector.tensor_sub(w2[:, :, :, 1, :], w1[:, :, :, 0, :], w1[:, :, :, 1, :])

                # stage 3: combine j-bit2 (element distance 32)
                d3 = work_pool.tile([P, F], dt.bfloat16, tag="d3")
                x2 = d2[:].rearrange("p (b s jl) -> p b s jl", s=2, jl=32)
                x3 = d3[:].rearrange("p (b s jl) -> p b s jl", s=2, jl=32)
                nc.vector.tensor_add(x3[:, :, 0, :], x2[:, :, 0, :], x2[:, :, 1, :])
                nc.vector.tensor_sub(x3[:, :, 1, :], x2[:, :, 0, :], x2[:, :, 1, :])

                # abs + per-partition running sum for this tile (ACT engine).
                # The elementwise out is a dump; reuse d2 (dead after stage 3).
                nc.scalar.activation(
                    out=d2[:],
                    in_=d3[:],
                    func=mybir.ActivationFunctionType.Abs,
                    accum_out=acc[:, t : t + 1],
                )

            accsum = acc_pool.tile([P, 1], dt.float32)
            nc.vector.tensor_reduce(
                out=accsum[:],
                in_=acc[:],
                axis=mybir.AxisListType.X,
                op=mybir.AluOpType.add,
            )
            nc.sync.dma_start(out=out_dram[:, :], in_=accsum[:])

    nc.compile()
    return nc


_NC_CACHE: bass.Bass | None = None


def _get_program() -> bass.Bass:
    global _NC_CACHE
    if _NC_CACHE is None:
        _NC_CACHE = _build_program()
    return _NC_CACHE


def run(original: np.ndarray, pred: np.ndarray, trace: bool = False, **kwargs):
    """Shard, run on 8 cores, return (scalar result, BassKernelResults)."""
    o = np.asarray(original, dtype=np.float32).reshape(N_CORES, ROWS, F)
    p = np.asarray(pred, dtype=np.float32).reshape(N_CORES, ROWS, F)
    x = np.concatenate([o, p], axis=2)  # [N_CORES, ROWS, 2F] row-interleaved
    in_maps = [{"x": x[i]} for i in range(N_CORES)]
    nc = _get_program()
    res = run_bass_kernel_spmd(
        nc, in_maps, core_ids=list(range(N_CORES)), trace=trace, **kwargs
    )
    total = np.float64(0.0)
    for r in res.results:
        total += r["out"].astype(np.float64).sum()
    return np.array(total, dtype=np.float32), res


def kernel(original: np.ndarray, pred: np.ndarray) -> np.ndarray:
    out, _ = run(original, pred, trace=False)
    return out
